# revision 13
# baseline (speedup 1.0000x reference)
"""Trainium2 Bass kernel for CharacterLevelSpectral.

Math: the reference embeds chars (x = char/255; emb = x*W + b broadcast over D),
FFTs along seq, zeroes mid frequencies (keeps lowest k=S/4 and highest k), IFFTs,
takes the real part.  The whole pipeline is linear along seq and the bias is
constant along seq (a constant's spectrum lives at f=0, which the low-pass
keeps), so

    out[b, s, d] = y[b, s] * W[d] + b[d],   y = lowpass(char/255)

and the FFT only needs to run on the (B, S) scalar signal, not (B, S, D).

y is computed per batch row with a factorized N1=128 x N2=64 Cooley-Tukey
FFT -> mask -> IFFT: small bf16 matmuls on the TensorEngine (char values
0..255 and the stage roundings are exact/benign in bf16; accumulation is
fp32 in PSUM) plus two elementwise fp32 twiddle stages on the VectorEngine,
packed as [re|im] half-tiles so each stage is 2 big multiplies + 2 combines.
The frequency mask only depends on f2 (k = 2048 = 16*128), so the
DFT_64/mask/IDFT_64 stage collapses into one precomputed 64x64 complex
matrix G.

The memory-bound part is materializing the (2, 8192, 256) fp32 output per
core (16.8 MB).  That broadcast (out_chunk = y_col x W + b) runs on the
TensorEngine as bf16 K=9 row-tiled matmuls: lhsT rows = 8 y-chunks + a ones
row, rhs = block-diagonal W replicas with a bias row (bias folded into the
matmul).  Matmul operands must sit on 32-aligned partition strips, so y
groups live at partition offsets {0,32,64,96} and the W/b constant is
replicated at the same offsets; consecutive matmuls alternate strips so
their LDWEIGHTS can pull ahead in the PE queue.  The inverse-FFT stage
emits y directly in that strip layout: its input tile has 32-column strips
whose 9th column is (S, 0, 0, ...) so the IDFT matmul produces an exact
ones row - no partition shuffles anywhere.  PSUM->SBUF copies of the
broadcast tiles alternate between VectorE and ScalarE; 1MB staging tiles
DMA out with 8KB-contiguous per-partition descriptors.

Startup-latency details: constants ride in two packed DRAM tensors (one
bf16, one fp32 - three DMAs total including the W/b block); the char load
goes on the scalar-engine HWDGE queue so it doesn't serialize behind them.

Sharding: batch dim across 8 cores (2 rows per core), no cross-core traffic.
"""

import ml_dtypes
import numpy as np

import concourse.bass as bass
import concourse.mybir as mybir
import concourse.tile as tile
from concourse import bacc
from concourse.bass_utils import run_bass_kernel_spmd

B, S, D = 16, 8192, 256
NCORES = 8
BPC = B // NCORES  # batches per core
N1, N2 = 128, 64   # S = N1 * N2
KLP = S // 4       # low-pass cutoff
NG = 8             # chunks per broadcast group (K = NG + 1)

F32 = mybir.dt.float32
BF16 = mybir.dt.bfloat16
I32 = mybir.dt.int32
MULT = mybir.AluOpType.mult
ADD = mybir.AluOpType.add
SUB = mybir.AluOpType.subtract

# packed bf16 matmul-constant block: name -> (row0, rows, col0, cols)
CB16_LAYOUT = {
    "m1re": (0, 128, 0, 128),
    "m1im": (0, 128, 128, 128),
    "m3re": (0, 128, 256, 128),
    "m3imn": (0, 128, 384, 128),
    "gre": (0, 64, 512, 64),
    "gim": (0, 64, 576, 64),
    "gimn": (0, 64, 640, 64),
}
CB16_COLS = 704
# packed fp32 twiddle block
CB32_LAYOUT = {
    "tw2p1": (0, 128, 0, 128),
    "tw2p2": (0, 128, 128, 128),
    "twtp1": (0, 64, 256, 256),
    "twtp2": (0, 64, 512, 256),
}
CB32_COLS = 768


def make_consts():
    """Input-independent DFT/twiddle constants, packed into two blocks."""
    n1 = np.arange(N1)
    n2 = np.arange(N2)
    C128 = np.cos(2 * np.pi * np.outer(n1, n1) / N1)
    S128 = np.sin(2 * np.pi * np.outer(n1, n1) / N1)
    kept = np.r_[0 : KLP // N1, N2 - KLP // N1 : N2]
    diff = n2[None, :] - n2[:, None]  # [n2, m2']: m2' - n2
    G = sum(np.exp(2j * np.pi * diff * f2 / N2) for f2 in kept)
    twtre = np.cos(2 * np.pi * np.outer(n2, n1) / S)    # [n2, f1]
    twtim = -np.sin(2 * np.pi * np.outer(n2, n1) / S)
    tw2re = np.cos(2 * np.pi * np.outer(n1, n2) / S)    # [f1, m2']
    tw2im = np.sin(2 * np.pi * np.outer(n1, n2) / S)
    c16 = {
        "m1re": C128 / 255.0,
        "m1im": -S128 / 255.0,
        "m3re": C128 / S,
        "m3imn": -S128 / S,
        "gre": G.real,
        "gim": G.imag,
        "gimn": -G.imag,
    }
    c32 = {
        "tw2p1": np.concatenate([tw2re, tw2im], axis=1),
        "tw2p2": np.concatenate([tw2im, tw2re], axis=1),
        "twtp1": np.concatenate([twtre, twtim], axis=1),
        "twtp2": np.concatenate([twtim, twtre], axis=1),
    }
    b16 = np.zeros((N1, CB16_COLS), dtype=np.float32)
    for name, (r0, rs, c0, cs) in CB16_LAYOUT.items():
        b16[r0 : r0 + rs, c0 : c0 + cs] = c16[name]
    b32 = np.zeros((N1, CB32_COLS), dtype=np.float32)
    for name, (r0, rs, c0, cs) in CB32_LAYOUT.items():
        b32[r0 : r0 + rs, c0 : c0 + cs] = c32[name]
    return b16.astype(ml_dtypes.bfloat16), b32


def build_program():
    """Build the per-core SPMD Bass program (identical on all cores)."""
    nc = bacc.Bacc("TRN2", target_bir_lowering=False, debug=False)

    # char values 0..255 are all exactly representable in bf16, so the host
    # passes them pre-cast (pure dtype marshaling) and MM1 consumes directly
    char_ext = nc.dram_tensor("char", [BPC, N1, N2], BF16, kind="ExternalInput").ap()
    # 4 strip-replicas of [block-diag W | bias row], bf16
    wb4_ext = nc.dram_tensor("wb4", [105, NG * D], BF16, kind="ExternalInput").ap()
    cb16_ext = nc.dram_tensor("cb16", [N1, CB16_COLS], BF16, kind="ExternalInput").ap()
    cb32_ext = nc.dram_tensor("cb32", [N1, CB32_COLS], F32, kind="ExternalInput").ap()
    # out[b, p, g, f] with s = 64*p + 8*g + f//256, d = f%256  — row-major
    # identical to (BPC, S, D)
    out_ext = nc.dram_tensor("out", [BPC, N1, 8, 2048], F32, kind="ExternalOutput").ap()

    with tile.TileContext(nc) as tc:
        with (
            tc.tile_pool(name="consts", bufs=1) as cpool,
            tc.tile_pool(name="work", bufs=2) as wpool,
            tc.tile_pool(name="stg", bufs=6) as spool,
            tc.tile_pool(name="ppfft", bufs=1, space="PSUM") as ppfft,
            tc.tile_pool(name="ppy", bufs=2, space="PSUM") as ppy,
            tc.tile_pool(name="ppb", bufs=5, space="PSUM") as ppb,
        ):
            # ---- input loads, all on the sync HWDGE queue: char first (it
            # heads the FFT dependency chain), then constants in use order ----
            xall = cpool.tile([N1, 2 * N2], BF16)
            nc.sync.dma_start(
                out=xall.rearrange("p (b n) -> p b n", b=BPC)[:],
                in_=char_ext.rearrange("b p n -> p b n")[:],
            )
            cb16 = cpool.tile([N1, CB16_COLS], BF16)
            nc.sync.dma_start(out=cb16[:], in_=cb16_ext)
            cb32 = cpool.tile([N1, CB32_COLS], F32)
            nc.sync.dma_start(out=cb32[:], in_=cb32_ext)
            wb4 = cpool.tile([105, NG * D], BF16)
            nc.sync.dma_start(out=wb4[:], in_=wb4_ext)
            cs = {
                name: cb16[r0 : r0 + rs, c0 : c0 + cc]
                for name, (r0, rs, c0, cc) in CB16_LAYOUT.items()
            }
            cs.update(
                {
                    name: cb32[r0 : r0 + rs, c0 : c0 + cc]
                    for name, (r0, rs, c0, cc) in CB32_LAYOUT.items()
                }
            )

            ylhs_batches = []
            for bb in range(BPC):
                xf = xall[:, bb * N2 : (bb + 1) * N2]

                # ---- MM1: A'[n2, f1] = Xm.T @ M1 (re | im packed in free) ----
                apack = ppfft.tile([N2, 2 * N1], F32, tag="fftps")
                are, aim = apack[:, 0:N1], apack[:, N1 : 2 * N1]
                nc.tensor.matmul(are, xf, cs["m1re"], start=True, stop=True)
                nc.tensor.matmul(aim, xf, cs["m1im"], start=True, stop=True)

                # ---- twiddle 1: B' = A' * TWT (complex, packed ops) ----
                #   u = [are*twtre | aim*twtim],  v = [are*twtim | aim*twtre]
                u = wpool.tile([N2, 2 * N1], F32, tag="u")
                nc.vector.tensor_tensor(u[:], apack[:], cs["twtp1"], MULT)
                v = wpool.tile([N2, 2 * N1], F32, tag="v")
                nc.vector.tensor_tensor(v[:], apack[:], cs["twtp2"], MULT)
                bre = wpool.tile([N2, N1], BF16, tag="bre")
                nc.vector.tensor_tensor(bre[:], u[:, 0:N1], u[:, N1 : 2 * N1], SUB)
                bim = wpool.tile([N2, N1], BF16, tag="bim")
                nc.vector.tensor_tensor(bim[:], v[:, 0:N1], v[:, N1 : 2 * N1], ADD)

                # ---- MM2: Ck[f1, m2'] = B'.T @ G (re | im packed in free) ----
                ckpack = ppfft.tile([N1, 2 * N2], F32, tag="fftps")
                ckre, ckim = ckpack[:, 0:N2], ckpack[:, N2 : 2 * N2]
                nc.tensor.matmul(ckre, bre[:], cs["gre"], start=True, stop=False)
                nc.tensor.matmul(ckre, bim[:], cs["gimn"], start=False, stop=True)
                nc.tensor.matmul(ckim, bre[:], cs["gim"], start=True, stop=False)
                nc.tensor.matmul(ckim, bim[:], cs["gre"], start=False, stop=True)

                # ---- twiddle 2: Dm = Ck * TW2, written into two (128,128)
                # bf16 tiles whose free dim is 4 strips of 32: [8 data cols |
                # ones col | 23 zero cols].  The ones col is (S,0,...) so MM3
                # emits an exact ones row on that output partition. ----
                #   u2 = [ckre*tw2re | ckim*tw2im], v2 = [ckre*tw2im | ckim*tw2re]
                u2 = wpool.tile([N1, 2 * N2], F32, tag="u2")
                nc.vector.tensor_tensor(u2[:], ckpack[:], cs["tw2p1"], MULT)
                v2 = wpool.tile([N1, 2 * N2], F32, tag="v2")
                nc.vector.tensor_tensor(v2[:], ckpack[:], cs["tw2p2"], MULT)

                ylhs_half = []
                for half in range(2):
                    dmre = wpool.tile([N1, 128], BF16, tag=f"dmre{half}")
                    dmim = wpool.tile([N1, 128], BF16, tag=f"dmim{half}")
                    re3 = dmre.rearrange("p (g n) -> p g n", n=32)
                    im3 = dmim.rearrange("p (g n) -> p g n", n=32)
                    nc.gpsimd.memset(re3[:, :, NG:32], 0.0)
                    nc.gpsimd.memset(im3[:, :, NG:32], 0.0)
                    nc.gpsimd.memset(re3[0:1, :, NG : NG + 1], float(S))
                    cols = slice(32 * half, 32 * half + 32)
                    colsi = slice(N2 + 32 * half, N2 + 32 * half + 32)
                    ua = u2[:, cols].rearrange("p (g c) -> p g c", c=NG)
                    ub = u2[:, colsi].rearrange("p (g c) -> p g c", c=NG)
                    nc.vector.tensor_tensor(re3[:, :, 0:NG], ua, ub, SUB)
                    va = v2[:, cols].rearrange("p (g c) -> p g c", c=NG)
                    vb = v2[:, colsi].rearrange("p (g c) -> p g c", c=NG)
                    nc.vector.tensor_tensor(im3[:, :, 0:NG], va, vb, ADD)

                    # ---- MM3: ylhs[32g+c, p] = y[64p + 8(4*half+g) + c],
                    # ylhs[32g+8, :] = 1 ----
                    ylhs_ps = ppy.tile([N1, N1], F32, tag="ylhs_ps")
                    nc.tensor.matmul(
                        ylhs_ps[:], dmre[:], cs["m3re"], start=True, stop=False
                    )
                    nc.tensor.matmul(
                        ylhs_ps[:], dmim[:], cs["m3imn"], start=False, stop=True
                    )
                    ylhs = wpool.tile([N1, N1], BF16, tag=f"ylhs{half}")
                    nc.vector.tensor_copy(ylhs[:], ylhs_ps[:])
                    ylhs_half.append(ylhs)
                ylhs_batches.append(ylhs_half)

            # ---- broadcast: K=9 bf16 row-tiled matmuls, bias folded; group
            # pairs interleaved so consecutive matmuls sit on different PE
            # row strips (LDWEIGHTS pulls ahead) ----
            for bb in range(BPC):
                ylhs_half = ylhs_batches[bb]
                for pair in range(4):
                    gs = (2 * pair, 2 * pair + 1)
                    stgs = {}
                    for g in gs:
                        stgs[g] = spool.tile(
                            [N1, NG * D], F32, tag="stg", name=f"stg{bb}_{g}"
                        )
                    for q in range(4):
                        for g in gs:
                            ylhs = ylhs_half[g // 4]
                            gp = 32 * (g % 4)  # partition strip
                            rows = slice(gp, gp + NG + 1)
                            bcps = ppb.tile([N1, 512], F32, tag="bcps")
                            nc.tensor.matmul(
                                bcps[:],
                                ylhs[rows, :],
                                wb4[rows, 512 * q : 512 * (q + 1)],
                                start=True,
                                stop=True,
                                tile_position=(gp, 0),
                            )
                            dst = stgs[g][:, 512 * q : 512 * (q + 1)]
                            if (q + g) % 2 == 0:
                                nc.scalar.copy(dst, bcps[:])
                            else:
                                nc.vector.tensor_copy(dst, bcps[:])
                    for g in gs:
                        nc.sync.dma_start(out=out_ext[bb, :, g, :], in_=stgs[g][:])

    nc.compile()
    return nc


_NC = None


def _get_nc():
    global _NC
    if _NC is None:
        _NC = build_program()
    return _NC


def make_in_maps(char_ids, W, b):
    char = np.asarray(char_ids).astype(np.float32).astype(ml_dtypes.bfloat16)
    char = np.ascontiguousarray(char).reshape(NCORES, BPC, N1, N2)
    wvec = np.asarray(W, dtype=np.float32)[:, 0]
    bvec = np.asarray(b, dtype=np.float32)
    wb9 = np.zeros((NG + 1, NG * D), dtype=np.float32)
    for c in range(NG):
        wb9[c, c * D : (c + 1) * D] = wvec
    wb9[NG] = np.tile(bvec, NG)
    wb4 = np.zeros((105, NG * D), dtype=np.float32)
    for g in range(4):
        wb4[32 * g : 32 * g + NG + 1] = wb9
    wb4 = wb4.astype(ml_dtypes.bfloat16)
    cb16, cb32 = make_consts()
    in_maps = []
    for i in range(NCORES):
        in_maps.append({"char": char[i], "wb4": wb4, "cb16": cb16, "cb32": cb32})
    return in_maps


def kernel(char_ids, W, b):
    nc = _get_nc()
    in_maps = make_in_maps(char_ids, W, b)
    res = run_bass_kernel_spmd(nc, in_maps, core_ids=list(range(NCORES)))
    parts = [r["out"].reshape(BPC, S, D) for r in res.results]
    return np.concatenate(parts, axis=0).astype(np.float32)
